# revision 4
# baseline (speedup 1.0000x reference)
"""CrystalFormer 4-layer dense transformer on 8 trn2 cores.

Sharding: DP=4 over batch (B=4 samples) x TP=2 over heads / d_ff
(Megatron column/row split). Core 2b+r handles sample b with TP rank r.
Pairs {2b, 2b+1} AllReduce partial outputs of the O-projection and FFN2.

All matmuls run in bf16 (fp32 PSUM accumulation); the residual stream,
layernorm statistics and softmax math stay fp32.  Softmax denominators
come for free from a ones-column appended to V (row 64 of the AV psum);
no max-subtraction is needed (scores are O(1) for this model family).
"""

import os
import sys
from contextlib import ExitStack

import numpy as np

try:
    import concourse.bass as bass  # noqa: F401
except Exception:  # pragma: no cover
    sys.path.insert(0, "/opt/trn_rl_repo")

import ml_dtypes

import concourse.bass as bass
import concourse.tile as tile
from concourse import bacc, mybir
from concourse.bass_utils import run_bass_kernel_spmd

F32 = mybir.dt.float32
BF16 = mybir.dt.bfloat16
AF = mybir.ActivationFunctionType
ALU = mybir.AluOpType

# model dims (hardcoded per problem spec)
L, S, D = 4, 1024, 1024
H, DK, DV = 16, 64, 64
DQ = 512          # per-rank head width (8 heads x 64)
FF = 2048         # per-rank d_ff
NP = 128          # partitions
SC = S // NP      # 8 s-chunks
DC = D // NP      # 8 d-chunks
QC = DQ // NP     # 4 dq chunks
FC = FF // NP     # 16 ff chunks
NH = 2            # n-halves of 512 over D
EPS = 1e-5

GROUPS = [[0, 1], [2, 3], [4, 5], [6, 7]]

LAST_RESULTS = None  # set by kernel(): BassKernelResults of the last run


def _bcast_ap(ap, parts=NP):
    """DRAM AP broadcast across partitions (stride-0 partition dim)."""
    return bass.AP(tensor=ap.tensor, offset=ap.offset, ap=[[0, parts]] + list(ap.ap))


def build_program(has_qkv_bias, has_o_bias, has_ffn_bias, has_ln1_aff,
                  has_ln2_aff, has_mask):
    nc = bacc.Bacc("TRN2", target_bir_lowering=False, debug=False,
                   num_devices=8)

    h0 = nc.dram_tensor("h0", [S, D], F32, kind="ExternalInput").ap()
    wq = nc.dram_tensor("wq", [L, D, DQ], BF16, kind="ExternalInput").ap()
    wk = nc.dram_tensor("wk", [L, D, DQ], BF16, kind="ExternalInput").ap()
    wv = nc.dram_tensor("wv", [L, D, DQ], BF16, kind="ExternalInput").ap()
    wo = nc.dram_tensor("wo", [L, DQ, D], BF16, kind="ExternalInput").ap()
    w1 = nc.dram_tensor("w1", [L, D, FF], BF16, kind="ExternalInput").ap()
    w2 = nc.dram_tensor("w2", [L, FF, D], BF16, kind="ExternalInput").ap()
    bq = bk = bv = bo2 = b1 = b22 = None
    g1 = be1 = g2 = be2 = maskT = None
    if has_qkv_bias:
        bq = nc.dram_tensor("bq", [L, DQ], F32, kind="ExternalInput").ap()
        bk = nc.dram_tensor("bk", [L, DQ], F32, kind="ExternalInput").ap()
        bv = nc.dram_tensor("bv", [L, DQ], F32, kind="ExternalInput").ap()
    if has_o_bias:
        bo2 = nc.dram_tensor("bo2", [L, D], F32, kind="ExternalInput").ap()
    if has_ffn_bias:
        b1 = nc.dram_tensor("b1", [L, FF], F32, kind="ExternalInput").ap()
        b22 = nc.dram_tensor("b22", [L, D], F32, kind="ExternalInput").ap()
    if has_ln1_aff:
        g1 = nc.dram_tensor("g1", [L, D], F32, kind="ExternalInput").ap()
        be1 = nc.dram_tensor("be1", [L, D], F32, kind="ExternalInput").ap()
    if has_ln2_aff:
        g2 = nc.dram_tensor("g2", [L, D], F32, kind="ExternalInput").ap()
        be2 = nc.dram_tensor("be2", [L, D], F32, kind="ExternalInput").ap()
    if has_mask:
        maskT = nc.dram_tensor("maskT", [S, S], F32, kind="ExternalInput").ap()

    out_ext = nc.dram_tensor("out", [S, D], F32, kind="ExternalOutput").ap()

    with tile.TileContext(nc) as tc, ExitStack() as ctx:
        p3 = ctx.enter_context(tc.tile_pool(name="p3", bufs=2))
        p4 = ctx.enter_context(tc.tile_pool(name="p4", bufs=4))
        p8 = ctx.enter_context(tc.tile_pool(name="p8", bufs=8))
        p16 = ctx.enter_context(tc.tile_pool(name="p16", bufs=16))
        p2 = ctx.enter_context(tc.tile_pool(name="p2", bufs=2))
        p1 = ctx.enter_context(tc.tile_pool(name="p1", bufs=1))
        psum = ctx.enter_context(tc.tile_pool(name="psum", bufs=8, space="PSUM"))
        dram = ctx.enter_context(tc.tile_pool(name="dram", bufs=6, space="DRAM"))
        hnd = ctx.enter_context(tc.tile_pool(name="hnd", bufs=2, space="DRAM"))

        eps_t = p1.tile([NP, 1], F32, tag="eps")
        nc.vector.memset(eps_t[:], EPS)

        def ln_block(src, g_ap, b_ap):
            """LayerNorm src [S, D] f32 -> hn (bf16, DRAM) -> hnT sbuf tiles."""
            if g_ap is not None:
                g_bc = p3.tile([NP, D], F32, tag="g_bc")
                nc.gpsimd.dma_start(out=g_bc[:], in_=_bcast_ap(g_ap))
                b_bc = p3.tile([NP, D], F32, tag="b_bc")
                nc.gpsimd.dma_start(out=b_bc[:], in_=_bcast_ap(b_ap))
            hn_dram = hnd.tile([S, D], BF16, tag="hn_d")
            for sc in range(SC):
                h_t = p3.tile([NP, D], F32, tag="h_t")
                nc.sync.dma_start(h_t[:], src[sc * NP:(sc + 1) * NP, :])
                stats = p3.tile([NP, 2, 6], F32, tag="bnst")
                for sub in range(2):
                    nc.vector.bn_stats(stats[:, sub, :],
                                       h_t[:, sub * 512:(sub + 1) * 512])
                mv = p3.tile([NP, 2], F32, tag="mv")
                nc.vector.bn_aggr(mv[:], stats[:])
                std = p3.tile([NP, 1], F32, tag="std")
                nc.scalar.activation(std[:], mv[:, 1:2], AF.Sqrt,
                                     bias=eps_t[:], scale=1.0)
                rstd = p3.tile([NP, 1], F32, tag="rstd")
                nc.vector.reciprocal(rstd[:], std[:])
                hn_t = p3.tile([NP, D], BF16, tag="hn_t")
                if g_ap is None:
                    nc.vector.tensor_scalar(
                        out=hn_t[:], in0=h_t[:], scalar1=mv[:, 0:1],
                        scalar2=rstd[:], op0=ALU.subtract, op1=ALU.mult)
                else:
                    tmp = p3.tile([NP, D], F32, tag="ln_tmp")
                    nc.vector.tensor_scalar(
                        out=tmp[:], in0=h_t[:], scalar1=mv[:, 0:1],
                        scalar2=rstd[:], op0=ALU.subtract, op1=ALU.mult)
                    nc.vector.tensor_mul(tmp[:], tmp[:], g_bc[:])
                    nc.vector.tensor_tensor(hn_t[:], tmp[:], b_bc[:], ALU.add)
                nc.sync.dma_start(hn_dram[sc * NP:(sc + 1) * NP, :], hn_t[:])
            hnT = []
            for j in range(DC):
                t = p8.tile([NP, S], BF16, tag="hnT")
                for sh in range(2):
                    nc.sync.dma_start(
                        out=t[:, sh * 512:(sh + 1) * 512],
                        in_=hn_dram[sh * 512:(sh + 1) * 512,
                                    j * NP:(j + 1) * NP],
                        transpose=True)
                hnT.append(t)
            return hnT

        def proj_qkT(i, w_ap, hnT, b_ap, out_tag):
            """qT/kT [DQ, S]: out[m][:, :] = (hn @ W)^T; bf16 tiles."""
            outs = [p4.tile([NP, S], BF16, tag=out_tag, name=out_tag) for _ in range(QC)]
            if b_ap is not None:
                b_sb = p3.tile([NP, QC], F32, tag=out_tag + "_b")
                nc.sync.dma_start(
                    b_sb[:], b_ap[i].rearrange("(c p) -> p c", p=NP))
            for grp in ((0, 1), (2, 3)):
                pss = {(m, nh): psum.tile([NP, 512], F32, tag="ps", name="ps")
                       for m in grp for nh in range(NH)}
                for j in range(DC):
                    w_t = p3.tile([NP, DQ], BF16, tag=out_tag + "_w")
                    nc.sync.dma_start(w_t[:], w_ap[i, j * NP:(j + 1) * NP, :])
                    for m in grp:
                        for nh in range(NH):
                            nc.tensor.matmul(
                                pss[(m, nh)][:],
                                w_t[:, m * NP:(m + 1) * NP],
                                hnT[j][:, nh * 512:(nh + 1) * 512],
                                start=(j == 0), stop=(j == DC - 1))
                for m in grp:
                    for nh in range(NH):
                        if b_ap is None:
                            nc.scalar.activation(
                                outs[m][:, nh * 512:(nh + 1) * 512],
                                pss[(m, nh)][:], AF.Copy)
                        else:
                            nc.scalar.activation(
                                outs[m][:, nh * 512:(nh + 1) * 512],
                                pss[(m, nh)][:], AF.Identity,
                                bias=b_sb[:, m:m + 1])
            return outs

        def proj_v(i, hnT, b_ap):
            """v_aug[sc] [128, 8, 65] bf16: v rows + ones column."""
            outs = []
            if b_ap is not None:
                bv_bc = p3.tile([NP, DQ], F32, tag="bv_bc")
                nc.gpsimd.dma_start(out=bv_bc[:], in_=_bcast_ap(b_ap[i]))
            vaug = [p8.tile([NP, H // 2, DV + 1], BF16, tag="vaug", name="vaug")
                    for _ in range(SC)]
            for grp in ((0, 1, 2, 3), (4, 5, 6, 7)):
                pss = {m: psum.tile([NP, 512], F32, tag="ps", name="ps") for m in grp}
                for j in range(DC):
                    w_t = p3.tile([NP, DQ], BF16, tag="wv_w")
                    nc.sync.dma_start(w_t[:], wv[i, j * NP:(j + 1) * NP, :])
                    for m in grp:
                        nc.tensor.matmul(
                            pss[m][:], hnT[j][:, m * NP:(m + 1) * NP], w_t[:],
                            start=(j == 0), stop=(j == DC - 1))
                for m in grp:
                    va = vaug[m]
                    nc.vector.memset(va[:, :, DV:DV + 1], 1.0)
                    src = pss[m][:].rearrange("p (h d) -> p h d", h=H // 2)
                    if b_ap is None:
                        nc.vector.tensor_copy(va[:, :, 0:DV], src)
                    else:
                        nc.vector.tensor_tensor(
                            va[:, :, 0:DV], src,
                            bv_bc[:].rearrange("p (h d) -> p h d", h=H // 2),
                            ALU.add)
                    outs.append(va)
            return vaug

        def attention(i, qT, kT, vaug):
            """attnT[p] [128, S] bf16 (normalized (attn@V)^T, 2 heads/pair)."""
            attnT = []
            for p in range(QC):
                at = p4.tile([NP, S], BF16, tag="attnT")
                pb = [p2.tile([NP, SC, S], BF16, tag="pb", name="pb") for _ in range(2)]
                for kc in range(SC):
                    for qh in range(NH):
                        for t in range(2):
                            b = 64 * t
                            ps_s = psum.tile([NP, 512], F32, tag="ps")
                            nc.tensor.matmul(
                                ps_s[:],
                                kT[p][b:b + 64, kc * NP:(kc + 1) * NP],
                                qT[p][b:b + 64, qh * 512:(qh + 1) * 512],
                                start=True, stop=True)
                            if maskT is not None:
                                mb_t = p3.tile([NP, 512], F32, tag="mb")
                                nc.sync.dma_start(
                                    mb_t[:],
                                    maskT[kc * NP:(kc + 1) * NP,
                                          qh * 512:(qh + 1) * 512])
                                nc.vector.tensor_tensor(
                                    ps_s[:], ps_s[:], mb_t[:], ALU.add)
                            nc.scalar.activation(
                                pb[t][:, kc, qh * 512:(qh + 1) * 512],
                                ps_s[:], AF.Exp, scale=1.0 / 8.0)
                for t in range(2):
                    b = 64 * t
                    for qh in range(NH):
                        ps_o = psum.tile([NP, 512], F32, tag="ps")
                        for kc in range(SC):
                            nc.tensor.matmul(
                                ps_o[0:DV + 1, :],
                                vaug[kc][:, 2 * p + t, :],
                                pb[t][:, kc, qh * 512:(qh + 1) * 512],
                                start=(kc == 0), stop=(kc == SC - 1))
                        rec = p3.tile([NP, 512], F32, tag="rec")
                        nc.vector.reciprocal(rec[DV:DV + 1, :],
                                             ps_o[DV:DV + 1, :])
                        r0 = p3.tile([1, 512], F32, tag="r0")
                        nc.sync.dma_start(r0[:], rec[DV:DV + 1, :])
                        rb = p3.tile([DV, 512], F32, tag="rb")
                        nc.gpsimd.partition_broadcast(rb[:], r0[:],
                                                      channels=DV)
                        if t == 0:
                            nc.vector.tensor_tensor(
                                at[0:DV, qh * 512:(qh + 1) * 512],
                                ps_o[0:DV, :], rb[:], ALU.mult)
                        else:
                            tmp = p3.tile([DV, 512], BF16, tag="at_tmp")
                            nc.vector.tensor_tensor(
                                tmp[:], ps_o[0:DV, :], rb[:], ALU.mult)
                            nc.sync.dma_start(
                                at[DV:NP, qh * 512:(qh + 1) * 512], tmp[:])
                attnT.append(at)
            return attnT

        def residual_out(ps_t, src, sc, nh, badd_bc, cc_dram):
            """cc = psum + 0.5*src_chunk (+ bias/2); DMA to cc_dram."""
            h_tmp = p3.tile([NP, 512], F32, tag="htmp")
            nc.sync.dma_start(h_tmp[:], src[sc * NP:(sc + 1) * NP,
                                            nh * 512:(nh + 1) * 512])
            hh_t = p3.tile([NP, 512], F32, tag="hh")
            nc.scalar.activation(hh_t[:], h_tmp[:], AF.Copy, scale=0.5)
            cc_t = p3.tile([NP, 512], F32, tag="cc_t")
            nc.vector.tensor_tensor(cc_t[:], ps_t[:], hh_t[:], ALU.add)
            if badd_bc is not None:
                nc.vector.tensor_tensor(
                    cc_t[:], cc_t[:],
                    badd_bc[:, nh * 512:(nh + 1) * 512], ALU.add)
            nc.sync.dma_start(
                cc_dram[sc * NP:(sc + 1) * NP, nh * 512:(nh + 1) * 512],
                cc_t[:])

        h_src = h0
        for i in range(L):
            # ---------- attention half ----------
            hnT = ln_block(h_src, g1[i] if g1 is not None else None,
                           be1[i] if be1 is not None else None)
            qT = proj_qkT(i, wq, hnT, bq, "qT")
            kT = proj_qkT(i, wk, hnT, bk, "kT")
            vaug = proj_v(i, hnT, bv)
            attnT = attention(i, qT, kT, vaug)

            wo_t = []
            for vc in range(QC):
                t = p4.tile([NP, D], BF16, tag="wo_w")
                nc.sync.dma_start(t[:], wo[i, vc * NP:(vc + 1) * NP, :])
                wo_t.append(t)
            bo_bc = None
            if bo2 is not None:
                bo_bc = p3.tile([NP, D], F32, tag="bo_bc")
                nc.gpsimd.dma_start(out=bo_bc[:], in_=_bcast_ap(bo2[i]))
            cc_attn_in = dram.tile([S, D], F32, tag="cc")
            cc_attn_out = dram.tile([S, D], F32, tag="cc")
            for scg in ((0, 1, 2, 3), (4, 5, 6, 7)):
                pss = {(sc, nh): psum.tile([NP, 512], F32, tag="ps", name="ps")
                       for sc in scg for nh in range(NH)}
                for vc in range(QC):
                    for sc in scg:
                        for nh in range(NH):
                            nc.tensor.matmul(
                                pss[(sc, nh)][:],
                                attnT[vc][:, sc * NP:(sc + 1) * NP],
                                wo_t[vc][:, nh * 512:(nh + 1) * 512],
                                start=(vc == 0), stop=(vc == QC - 1))
                for sc in scg:
                    for nh in range(NH):
                        residual_out(pss[(sc, nh)][:], h_src, sc, nh,
                                     bo_bc, cc_attn_in)
            nc.gpsimd.collective_compute(
                "AllReduce", ALU.add, replica_groups=GROUPS,
                ins=[cc_attn_in.opt()], outs=[cc_attn_out.opt()])

            # ---------- FFN half ----------
            hnT2 = ln_block(cc_attn_out, g2[i] if g2 is not None else None,
                            be2[i] if be2 is not None else None)
            w1_t = []
            for j in range(DC):
                t = p8.tile([NP, FF], BF16, tag="w1_w")
                nc.sync.dma_start(t[:], w1[i, j * NP:(j + 1) * NP, :])
                w1_t.append(t)
            b1_sb = None
            if b1 is not None:
                b1_sb = p3.tile([NP, FC], F32, tag="b1_sb")
                nc.sync.dma_start(b1_sb[:],
                                  b1[i].rearrange("(c p) -> p c", p=NP))
            aT = [p16.tile([NP, S], BF16, tag="aT", name="aT") for _ in range(FC)]
            for mg in range(4):
                ms = range(mg * 4, mg * 4 + 4)
                pss = {(m, nh): psum.tile([NP, 512], F32, tag="ps", name="ps")
                       for m in ms for nh in range(NH)}
                for j in range(DC):
                    for m in ms:
                        for nh in range(NH):
                            nc.tensor.matmul(
                                pss[(m, nh)][:],
                                w1_t[j][:, m * NP:(m + 1) * NP],
                                hnT2[j][:, nh * 512:(nh + 1) * 512],
                                start=(j == 0), stop=(j == DC - 1))
                for m in ms:
                    for nh in range(NH):
                        nc.scalar.activation(
                            aT[m][:, nh * 512:(nh + 1) * 512],
                            pss[(m, nh)][:], AF.Gelu,
                            bias=(b1_sb[:, m:m + 1] if b1_sb is not None
                                  else 0.0))

            b2_bc = None
            if b22 is not None:
                b2_bc = p3.tile([NP, D], F32, tag="b2_bc")
                nc.gpsimd.dma_start(out=b2_bc[:], in_=_bcast_ap(b22[i]))
            cc_ffn_in = dram.tile([S, D], F32, tag="cc")
            cc_ffn_out = dram.tile([S, D], F32, tag="cc")
            for scg in ((0, 1, 2, 3), (4, 5, 6, 7)):
                pss = {(sc, nh): psum.tile([NP, 512], F32, tag="ps", name="ps")
                       for sc in scg for nh in range(NH)}
                for fc in range(FC):
                    w2_t = p3.tile([NP, D], BF16, tag="w2_w")
                    nc.sync.dma_start(w2_t[:], w2[i, fc * NP:(fc + 1) * NP, :])
                    for sc in scg:
                        for nh in range(NH):
                            nc.tensor.matmul(
                                pss[(sc, nh)][:],
                                aT[fc][:, sc * NP:(sc + 1) * NP],
                                w2_t[:, nh * 512:(nh + 1) * 512],
                                start=(fc == 0), stop=(fc == FC - 1))
                for sc in scg:
                    for nh in range(NH):
                        residual_out(pss[(sc, nh)][:], cc_attn_out, sc, nh,
                                     b2_bc, cc_ffn_in)
            nc.gpsimd.collective_compute(
                "AllReduce", ALU.add, replica_groups=GROUPS,
                ins=[cc_ffn_in.opt()], outs=[cc_ffn_out.opt()])
            h_src = cc_ffn_out

        nc.sync.dma_start(out_ext[:], h_src[:])

    nc.compile()
    return nc


def kernel(h, mask, Wq, bq, Wk, bk, Wv, bv, Wo, bo,
           ln1_g, ln1_b, ln2_g, ln2_b, W1, b1, W2, b2):
    global LAST_RESULTS
    h = np.asarray(h, dtype=np.float32)
    mask = np.asarray(mask)
    f32 = lambda a: np.asarray(a, dtype=np.float32)
    bf = lambda a: np.asarray(a, dtype=np.float32).astype(ml_dtypes.bfloat16)

    Wq, Wk, Wv, Wo, W1, W2 = map(f32, (Wq, Wk, Wv, Wo, W1, W2))
    bq, bk, bv, bo, b1, b2 = map(f32, (bq, bk, bv, bo, b1, b2))
    ln1_g, ln1_b, ln2_g, ln2_b = map(f32, (ln1_g, ln1_b, ln2_g, ln2_b))

    has_qkv_bias = bool(np.any(bq) or np.any(bk) or np.any(bv))
    has_o_bias = bool(np.any(bo))
    has_ffn_bias = bool(np.any(b1) or np.any(b2))
    has_ln1_aff = not (np.all(ln1_g == 1.0) and not np.any(ln1_b))
    has_ln2_aff = not (np.all(ln2_g == 1.0) and not np.any(ln2_b))
    has_mask = bool(np.any(mask == 0))

    nc = build_program(has_qkv_bias, has_o_bias, has_ffn_bias,
                       has_ln1_aff, has_ln2_aff, has_mask)

    in_maps = []
    for core in range(8):
        b, r = core // 2, core % 2
        c0, c1 = r * DQ, (r + 1) * DQ
        f0, f1 = r * FF, (r + 1) * FF
        m = {
            "h0": np.ascontiguousarray(h[b]),
            "wq": bf(Wq[:, :, c0:c1]),
            "wk": bf(Wk[:, :, c0:c1]),
            "wv": bf(Wv[:, :, c0:c1]),
            "wo": bf(Wo[:, c0:c1, :]),
            "w1": bf(W1[:, :, f0:f1]),
            "w2": bf(W2[:, f0:f1, :]),
        }
        if has_qkv_bias:
            m["bq"] = np.ascontiguousarray(bq[:, c0:c1])
            m["bk"] = np.ascontiguousarray(bk[:, c0:c1])
            m["bv"] = np.ascontiguousarray(bv[:, c0:c1])
        if has_o_bias:
            m["bo2"] = np.ascontiguousarray(bo * 0.5)
        if has_ffn_bias:
            m["b1"] = np.ascontiguousarray(b1[:, f0:f1])
            m["b22"] = np.ascontiguousarray(b2 * 0.5)
        if has_ln1_aff:
            m["g1"] = ln1_g
            m["be1"] = ln1_b
        if has_ln2_aff:
            m["g2"] = ln2_g
            m["be2"] = ln2_b
        if has_mask:
            mt = np.where(np.asarray(mask) == 0, np.float32(-1e9),
                          np.float32(0.0)).T
            m["maskT"] = np.ascontiguousarray(mt)
        in_maps.append(m)

    trace = bool(os.environ.get("KBENCH_TRACE"))
    tmpdir = os.environ.get("KBENCH_TMPDIR") or None
    res = run_bass_kernel_spmd(nc, in_maps, list(range(8)), trace=trace,
                               tmpdir=tmpdir)
    LAST_RESULTS = res
    out = np.stack([res.results[2 * b]["out"] for b in range(4)], axis=0)
    return out.astype(np.float32)


# revision 8
# speedup vs baseline: 1.3747x; 1.3747x over previous
"""CrystalFormer 4-layer dense transformer on 8 trn2 cores.

Sharding: DP=4 over batch (B=4 samples) x SP=2 over sequence.  Core
2b+r owns rows [r*512, (r+1)*512) of sample b.  Every projection /
FFN / layernorm / residual is purely local to those rows; the only
communication is one bf16 AllGather of K^T and V-augmented per layer
(so each core sees all 1024 key/value positions for its 512 queries).

The residual stream lives in SBUF in fp32 for the whole kernel; all
matmuls run in bf16 with fp32 PSUM accumulation.  hn -> hn^T uses the
TensorE transpose (identity matmul) - the DMA xbar transpose is
avoided entirely because Tile serializes it against in-flight
collectives (known HW hang workaround), which would stall the
pipeline.  Softmax denominators come free from a ones-column appended
to V (row 64 of the AV psum); no max-subtraction is needed (scores
are O(1) for this model family).
"""

import os
import sys
from contextlib import ExitStack

import numpy as np

try:
    import concourse.bass as bass  # noqa: F401
except Exception:  # pragma: no cover
    sys.path.insert(0, "/opt/trn_rl_repo")

import ml_dtypes

import concourse.bass as bass
import concourse.tile as tile
from concourse import bacc, mybir
from concourse.bass_utils import run_bass_kernel_spmd
from concourse.masks import make_identity

F32 = mybir.dt.float32
BF16 = mybir.dt.bfloat16
AF = mybir.ActivationFunctionType
ALU = mybir.AluOpType

# model dims (hardcoded per problem spec)
L, S, D = 4, 1024, 1024
H, DK, DV = 16, 64, 64
HD = H * DK       # 1024 = full head width
FF = 4096         # full d_ff
NP = 128
SL = 512          # local sequence rows per core
SCL = SL // NP    # 4 local s-chunks
DC = D // NP      # 8 d-chunks
HC = HD // NP     # 8 head-dim chunks (= head pairs)
FC = FF // NP     # 32 ff chunks
KC = S // NP      # 8 key-position chunks (global)
NH = 2            # 512-column halves of D
EPS = 1e-5

# flat bf16 element offsets inside the kv gather buffer
KT_ELEMS = HC * NP * SL            # 8 * 128 * 512
V_ELEMS = SCL * NP * H * (DV + 1)  # 4 * 128 * 16 * 65
KV_ELEMS = KT_ELEMS + V_ELEMS

GROUPS = [[0, 1], [2, 3], [4, 5], [6, 7]]

LAST_RESULTS = None  # set by kernel(): BassKernelResults of the last run


def _bcast_ap(ap, parts=NP):
    """DRAM AP broadcast across partitions (stride-0 partition dim)."""
    return bass.AP(tensor=ap.tensor, offset=ap.offset, ap=[[0, parts]] + list(ap.ap))


def build_program(has_qkv_bias, has_o_bias, has_ffn_bias, has_ln1_aff,
                  has_ln2_aff, has_mask):
    nc = bacc.Bacc("TRN2", target_bir_lowering=False, debug=False,
                   num_devices=8)

    h0 = nc.dram_tensor("h0", [SL, D], F32, kind="ExternalInput").ap()
    wq = nc.dram_tensor("wq", [L, D, HD], BF16, kind="ExternalInput").ap()
    wk = nc.dram_tensor("wk", [L, D, HD], BF16, kind="ExternalInput").ap()
    wv = nc.dram_tensor("wv", [L, D, HD], BF16, kind="ExternalInput").ap()
    wo = nc.dram_tensor("wo", [L, HD, D], BF16, kind="ExternalInput").ap()
    w1 = nc.dram_tensor("w1", [L, D, FF], BF16, kind="ExternalInput").ap()
    w2 = nc.dram_tensor("w2", [L, FF, D], BF16, kind="ExternalInput").ap()
    bq = bk = bv = bo = b1 = b2 = None
    g1 = be1 = g2 = be2 = maskT = None
    if has_qkv_bias:
        bq = nc.dram_tensor("bq", [L, HD], F32, kind="ExternalInput").ap()
        bk = nc.dram_tensor("bk", [L, HD], F32, kind="ExternalInput").ap()
        bv = nc.dram_tensor("bv", [L, HD], F32, kind="ExternalInput").ap()
    if has_o_bias:
        bo = nc.dram_tensor("bo", [L, D], F32, kind="ExternalInput").ap()
    if has_ffn_bias:
        b1 = nc.dram_tensor("b1", [L, FF], F32, kind="ExternalInput").ap()
        b2 = nc.dram_tensor("b2", [L, D], F32, kind="ExternalInput").ap()
    if has_ln1_aff:
        g1 = nc.dram_tensor("g1", [L, D], F32, kind="ExternalInput").ap()
        be1 = nc.dram_tensor("be1", [L, D], F32, kind="ExternalInput").ap()
    if has_ln2_aff:
        g2 = nc.dram_tensor("g2", [L, D], F32, kind="ExternalInput").ap()
        be2 = nc.dram_tensor("be2", [L, D], F32, kind="ExternalInput").ap()
    if has_mask:
        # per-core slice: [k_pos global, q local] additive bias
        maskT = nc.dram_tensor("maskT", [S, SL], F32,
                               kind="ExternalInput").ap()

    out_ext = nc.dram_tensor("out", [SL, D], F32, kind="ExternalOutput").ap()

    with tile.TileContext(nc) as tc, ExitStack() as ctx:
        p2 = ctx.enter_context(tc.tile_pool(name="p2", bufs=2))
        p3 = ctx.enter_context(tc.tile_pool(name="p3", bufs=3))
        p4 = ctx.enter_context(tc.tile_pool(name="p4", bufs=4))
        p8 = ctx.enter_context(tc.tile_pool(name="p8", bufs=8))
        p32 = ctx.enter_context(tc.tile_pool(name="p32", bufs=32))
        p1 = ctx.enter_context(tc.tile_pool(name="p1", bufs=1))
        psum = ctx.enter_context(tc.tile_pool(name="psum", bufs=8,
                                              space="PSUM"))
        dram = ctx.enter_context(tc.tile_pool(name="dram", bufs=4,
                                              space="DRAM"))

        eps_t = p1.tile([NP, 1], F32, tag="eps")
        nc.vector.memset(eps_t[:], EPS)
        ident = p1.tile([NP, NP], BF16, tag="ident")
        make_identity(nc, ident[:])

        # residual stream, fp32, SBUF-resident: 4 tiles of [128, D]
        h_res = []
        for c in range(SCL):
            t = p8.tile([NP, D], F32, tag="h", name="h")
            nc.sync.dma_start(t[:], h0[c * NP:(c + 1) * NP, :])
            h_res.append(t)

        def ln_T(h_tiles, g_bc, b_bc):
            """LayerNorm rows of h_tiles -> transposed bf16 hnT[j] [128, SL]."""
            hnT = [p8.tile([NP, SL], BF16, tag="hnT", name="hnT")
                   for _ in range(DC)]
            for c in range(SCL):
                h_t = h_tiles[c]
                stats = p3.tile([NP, 2, 6], F32, tag="bnst")
                for sub in range(2):
                    nc.vector.bn_stats(stats[:, sub, :],
                                       h_t[:, sub * 512:(sub + 1) * 512])
                mv = p3.tile([NP, 2], F32, tag="mv")
                nc.vector.bn_aggr(mv[:], stats[:])
                std = p3.tile([NP, 1], F32, tag="std")
                nc.scalar.activation(std[:], mv[:, 1:2], AF.Sqrt,
                                     bias=eps_t[:], scale=1.0)
                rstd = p3.tile([NP, 1], F32, tag="rstd")
                nc.vector.reciprocal(rstd[:], std[:])
                hn_t = p2.tile([NP, D], BF16, tag="hn_t")
                if g_bc is None:
                    nc.vector.tensor_scalar(
                        out=hn_t[:], in0=h_t[:], scalar1=mv[:, 0:1],
                        scalar2=rstd[:], op0=ALU.subtract, op1=ALU.mult)
                else:
                    tmp = p2.tile([NP, D], F32, tag="ln_tmp")
                    nc.vector.tensor_scalar(
                        out=tmp[:], in0=h_t[:], scalar1=mv[:, 0:1],
                        scalar2=rstd[:], op0=ALU.subtract, op1=ALU.mult)
                    nc.vector.tensor_mul(tmp[:], tmp[:], g_bc[:])
                    nc.vector.tensor_tensor(hn_t[:], tmp[:], b_bc[:], ALU.add)
                for j in range(DC):
                    pt = psum.tile([NP, NP], BF16, tag="ps", name="pt")
                    nc.tensor.transpose(pt[:], hn_t[:, j * NP:(j + 1) * NP],
                                        ident[:])
                    if j % 2 == 0:
                        nc.scalar.activation(
                            hnT[j][:, c * NP:(c + 1) * NP], pt[:], AF.Copy)
                    else:
                        nc.vector.tensor_copy(
                            hnT[j][:, c * NP:(c + 1) * NP], pt[:])
            return hnT

        def ln_aff_tiles(g_ap, b_ap):
            if g_ap is None:
                return None, None
            g_bc = p3.tile([NP, D], F32, tag="g_bc")
            nc.gpsimd.dma_start(out=g_bc[:], in_=_bcast_ap(g_ap))
            b_bc = p3.tile([NP, D], F32, tag="b_bc")
            nc.gpsimd.dma_start(out=b_bc[:], in_=_bcast_ap(b_ap))
            return g_bc, b_bc

        def proj_T(i, w_ap, hnT, b_ap, out_tag):
            """outT [HD, SL] = (hn @ W)^T as 8 bf16 tiles [128, SL]."""
            outs = [p8.tile([NP, SL], BF16, tag=out_tag, name=out_tag)
                    for _ in range(HC)]
            b_sb = None
            if b_ap is not None:
                b_sb = p3.tile([NP, HC], F32, tag=out_tag + "_b")
                nc.sync.dma_start(b_sb[:],
                                  b_ap[i].rearrange("(c p) -> p c", p=NP))
            for grp in (range(0, 4), range(4, 8)):
                pss = {m: psum.tile([NP, 512], F32, tag="ps", name="ps")
                       for m in grp}
                for j in range(DC):
                    w_t = p2.tile([NP, HD], BF16, tag=out_tag + "_w")
                    nc.sync.dma_start(w_t[:], w_ap[i, j * NP:(j + 1) * NP, :])
                    for m in grp:
                        nc.tensor.matmul(
                            pss[m][:], w_t[:, m * NP:(m + 1) * NP],
                            hnT[j][:], start=(j == 0), stop=(j == DC - 1))
                for m in grp:
                    if b_sb is None:
                        nc.scalar.activation(outs[m][:], pss[m][:], AF.Copy)
                    else:
                        nc.scalar.activation(outs[m][:], pss[m][:],
                                             AF.Identity,
                                             bias=b_sb[:, m:m + 1])
            return outs

        def proj_v(i, hnT, b_ap):
            """v_aug[c] [128, 16, 65] bf16 for local rows + ones column."""
            bv_bc = None
            if b_ap is not None:
                bv_bc = p3.tile([NP, HD], F32, tag="bv_bc")
                nc.gpsimd.dma_start(out=bv_bc[:], in_=_bcast_ap(b_ap[i]))
            vaug = [p4.tile([NP, H, DV + 1], BF16, tag="vaug", name="vaug")
                    for _ in range(SCL)]
            pss = {(c, dh): psum.tile([NP, 512], F32, tag="ps", name="ps")
                   for c in range(SCL) for dh in range(2)}
            for j in range(DC):
                w_t = p2.tile([NP, HD], BF16, tag="wv_w")
                nc.sync.dma_start(w_t[:], wv[i, j * NP:(j + 1) * NP, :])
                for c in range(SCL):
                    for dh in range(2):
                        nc.tensor.matmul(
                            pss[(c, dh)][:],
                            hnT[j][:, c * NP:(c + 1) * NP],
                            w_t[:, dh * 512:(dh + 1) * 512],
                            start=(j == 0), stop=(j == DC - 1))
            for c in range(SCL):
                va = vaug[c]
                nc.vector.memset(va[:, :, DV:DV + 1], 1.0)
                for dh in range(2):
                    src = pss[(c, dh)][:].rearrange("p (h d) -> p h d", h=8)
                    dst = va[:, dh * 8:(dh + 1) * 8, 0:DV]
                    if bv_bc is None:
                        nc.vector.tensor_copy(dst, src)
                    else:
                        nc.vector.tensor_tensor(
                            dst, src,
                            bv_bc[:, dh * 512:(dh + 1) * 512].rearrange(
                                "p (h d) -> p h d", h=8),
                            ALU.add)
            return vaug

        def kv_gather(i, kT, vaug):
            """AllGather local K^T and V_aug over the pair -> kv_out dram."""
            kv_in = dram.tile([KV_ELEMS], BF16, tag="kv_in", name="kv_in")
            kv_out = dram.tile([2 * KV_ELEMS], BF16, tag="kv_out",
                               name="kv_out")
            for j in range(HC):
                nc.sync.dma_start(
                    kv_in[j * NP * SL:(j + 1) * NP * SL].rearrange(
                        "(p f) -> p f", p=NP), kT[j][:])
            vsz = NP * H * (DV + 1)
            for c in range(SCL):
                nc.sync.dma_start(
                    kv_in[KT_ELEMS + c * vsz:KT_ELEMS + (c + 1) * vsz]
                    .rearrange("(p f) -> p f", p=NP),
                    vaug[c][:].rearrange("p h d -> p (h d)"))
            nc.gpsimd.collective_compute(
                "AllGather", ALU.bypass, replica_groups=GROUPS,
                ins=[kv_in.opt()], outs=[kv_out.opt()])
            return kv_out

        def attention(i, qT, kv_out):
            """attnT[p] [128, SL] bf16: normalized (attn @ V)^T, 2 heads."""
            vfull = []
            vsz = NP * H * (DV + 1)
            for kc in range(KC):
                slab, c = divmod(kc, SCL)
                off = slab * KV_ELEMS + KT_ELEMS + c * vsz
                t = p8.tile([NP, H, DV + 1], BF16, tag="vfull", name="vfull")
                nc.sync.dma_start(
                    t[:].rearrange("p h d -> p (h d)"),
                    kv_out[off:off + vsz].rearrange("(p f) -> p f", p=NP))
                vfull.append(t)
            attnT = []
            for p in range(HC):
                ktf = p4.tile([NP, S], BF16, tag="ktf", name="ktf")
                for slab in range(2):
                    off = slab * KV_ELEMS + p * NP * SL
                    nc.sync.dma_start(
                        ktf[:, slab * SL:(slab + 1) * SL],
                        kv_out[off:off + NP * SL].rearrange(
                            "(p f) -> p f", p=NP))
                at = p8.tile([NP, SL], BF16, tag="attnT", name="attnT")
                pb = [p2.tile([NP, KC, SL], BF16, tag="pb", name="pb")
                      for _ in range(2)]
                for kc in range(KC):
                    for t in range(2):
                        b = 64 * t
                        ps_s = psum.tile([NP, 512], F32, tag="ps", name="ps")
                        nc.tensor.matmul(
                            ps_s[:],
                            ktf[b:b + 64, kc * NP:(kc + 1) * NP],
                            qT[p][b:b + 64, :],
                            start=True, stop=True)
                        if maskT is not None:
                            mb_t = p3.tile([NP, SL], F32, tag="mb")
                            nc.sync.dma_start(
                                mb_t[:], maskT[kc * NP:(kc + 1) * NP, :])
                            nc.vector.tensor_tensor(ps_s[:], ps_s[:],
                                                    mb_t[:], ALU.add)
                        nc.scalar.activation(pb[t][:, kc, :], ps_s[:],
                                             AF.Exp, scale=1.0 / 8.0)
                for t in range(2):
                    ps_o = psum.tile([NP, 512], F32, tag="ps", name="ps")
                    for kc in range(KC):
                        nc.tensor.matmul(
                            ps_o[0:DV + 1, :],
                            vfull[kc][:, 2 * p + t, :],
                            pb[t][:, kc, :],
                            start=(kc == 0), stop=(kc == KC - 1))
                    rec = p2.tile([NP, SL], F32, tag="rec")
                    nc.vector.reciprocal(rec[DV:DV + 1, :],
                                         ps_o[DV:DV + 1, :])
                    r0 = p3.tile([1, SL], F32, tag="r0")
                    nc.sync.dma_start(r0[:], rec[DV:DV + 1, :])
                    rb = p3.tile([DV, SL], F32, tag="rb")
                    nc.gpsimd.partition_broadcast(rb[:], r0[:], channels=DV)
                    if t == 0:
                        nc.vector.tensor_tensor(at[0:DV, :], ps_o[0:DV, :],
                                                rb[:], ALU.mult)
                    else:
                        tmp = p3.tile([DV, SL], BF16, tag="at_tmp")
                        nc.vector.tensor_tensor(tmp[:], ps_o[0:DV, :],
                                                rb[:], ALU.mult)
                        nc.sync.dma_start(at[DV:NP, :], tmp[:])
                attnT.append(at)
            return attnT

        def add_residual(pss, badd_bc, h_old):
            """h_new[c] = psum[c,nh] + h_old[c] (+bias); returns new tiles."""
            h_new = []
            for c in range(SCL):
                t = p8.tile([NP, D], F32, tag="h", name="h")
                for nh in range(NH):
                    sl = slice(nh * 512, (nh + 1) * 512)
                    nc.vector.tensor_tensor(t[:, sl], pss[(c, nh)][:],
                                            h_old[c][:, sl], ALU.add)
                    if badd_bc is not None:
                        nc.vector.tensor_tensor(t[:, sl], t[:, sl],
                                                badd_bc[:, sl], ALU.add)
                h_new.append(t)
            return h_new

        for i in range(L):
            # ---------- attention ----------
            g_bc, b_bc = ln_aff_tiles(g1[i] if g1 is not None else None,
                                      be1[i] if be1 is not None else None)
            hnT = ln_T(h_res, g_bc, b_bc)
            kT = proj_T(i, wk, hnT, bk, "kT")
            vaug = proj_v(i, hnT, bv)
            kv_out = kv_gather(i, kT, vaug)
            qT = proj_T(i, wq, hnT, bq, "qT")
            attnT = attention(i, qT, kv_out)

            bo_bc = None
            if bo is not None:
                bo_bc = p3.tile([NP, D], F32, tag="bo_bc")
                nc.gpsimd.dma_start(out=bo_bc[:], in_=_bcast_ap(bo[i]))
            pss = {(c, nh): psum.tile([NP, 512], F32, tag="ps", name="ps")
                   for c in range(SCL) for nh in range(NH)}
            for vc in range(HC):
                wo_t = p3.tile([NP, D], BF16, tag="wo_w")
                nc.sync.dma_start(wo_t[:], wo[i, vc * NP:(vc + 1) * NP, :])
                for c in range(SCL):
                    for nh in range(NH):
                        nc.tensor.matmul(
                            pss[(c, nh)][:],
                            attnT[vc][:, c * NP:(c + 1) * NP],
                            wo_t[:, nh * 512:(nh + 1) * 512],
                            start=(vc == 0), stop=(vc == HC - 1))
            h_mid = add_residual(pss, bo_bc, h_res)

            # ---------- FFN ----------
            g_bc2, b_bc2 = ln_aff_tiles(g2[i] if g2 is not None else None,
                                        be2[i] if be2 is not None else None)
            hnT2 = ln_T(h_mid, g_bc2, b_bc2)
            b1_sb = None
            if b1 is not None:
                b1_sb = p3.tile([NP, FC], F32, tag="b1_sb")
                nc.sync.dma_start(b1_sb[:],
                                  b1[i].rearrange("(c p) -> p c", p=NP))
            aT = [p32.tile([NP, SL], BF16, tag="aT", name="aT")
                  for _ in range(FC)]
            for fq in range(4):
                ms = range(fq * 8, fq * 8 + 8)
                pss = {m: psum.tile([NP, 512], F32, tag="ps", name="ps")
                       for m in ms}
                for j in range(DC):
                    w_t = p3.tile([NP, 1024], BF16, tag="w1_w")
                    nc.sync.dma_start(
                        w_t[:], w1[i, j * NP:(j + 1) * NP,
                                   fq * 1024:(fq + 1) * 1024])
                    for mi, m in enumerate(ms):
                        nc.tensor.matmul(
                            pss[m][:], w_t[:, mi * NP:(mi + 1) * NP],
                            hnT2[j][:], start=(j == 0), stop=(j == DC - 1))
                for m in ms:
                    nc.scalar.activation(
                        aT[m][:], pss[m][:], AF.Gelu,
                        bias=(b1_sb[:, m:m + 1] if b1_sb is not None
                              else 0.0))

            b2_bc = None
            if b2 is not None:
                b2_bc = p3.tile([NP, D], F32, tag="b2_bc")
                nc.gpsimd.dma_start(out=b2_bc[:], in_=_bcast_ap(b2[i]))
            pss = {(c, nh): psum.tile([NP, 512], F32, tag="ps", name="ps")
                   for c in range(SCL) for nh in range(NH)}
            for fc in range(FC):
                w2_t = p3.tile([NP, D], BF16, tag="w2_w")
                nc.sync.dma_start(w2_t[:], w2[i, fc * NP:(fc + 1) * NP, :])
                for c in range(SCL):
                    for nh in range(NH):
                        nc.tensor.matmul(
                            pss[(c, nh)][:],
                            aT[fc][:, c * NP:(c + 1) * NP],
                            w2_t[:, nh * 512:(nh + 1) * 512],
                            start=(fc == 0), stop=(fc == FC - 1))
            h_res = add_residual(pss, b2_bc, h_mid)

        for c in range(SCL):
            nc.sync.dma_start(out_ext[c * NP:(c + 1) * NP, :], h_res[c][:])

    nc.compile()
    return nc


def kernel(h, mask, Wq, bq, Wk, bk, Wv, bv, Wo, bo,
           ln1_g, ln1_b, ln2_g, ln2_b, W1, b1, W2, b2):
    global LAST_RESULTS
    h = np.asarray(h, dtype=np.float32)
    mask = np.asarray(mask)
    f32 = lambda a: np.ascontiguousarray(np.asarray(a, dtype=np.float32))
    bf = lambda a: np.asarray(a, dtype=np.float32).astype(ml_dtypes.bfloat16)

    Wq, Wk, Wv, Wo, W1, W2 = map(f32, (Wq, Wk, Wv, Wo, W1, W2))
    bq, bk, bv, bo, b1, b2 = map(f32, (bq, bk, bv, bo, b1, b2))
    ln1_g, ln1_b, ln2_g, ln2_b = map(f32, (ln1_g, ln1_b, ln2_g, ln2_b))

    has_qkv_bias = bool(np.any(bq) or np.any(bk) or np.any(bv))
    has_o_bias = bool(np.any(bo))
    has_ffn_bias = bool(np.any(b1) or np.any(b2))
    has_ln1_aff = not (np.all(ln1_g == 1.0) and not np.any(ln1_b))
    has_ln2_aff = not (np.all(ln2_g == 1.0) and not np.any(ln2_b))
    has_mask = bool(np.any(mask == 0))

    nc = build_program(has_qkv_bias, has_o_bias, has_ffn_bias,
                       has_ln1_aff, has_ln2_aff, has_mask)

    wq_b, wk_b, wv_b = bf(Wq), bf(Wk), bf(Wv)
    wo_b, w1_b, w2_b = bf(Wo), bf(W1), bf(W2)
    mt = None
    if has_mask:
        mt = np.ascontiguousarray(
            np.where(mask == 0, np.float32(-1e9), np.float32(0.0)).T)

    in_maps = []
    for core in range(8):
        b, r = core // 2, core % 2
        m = {
            "h0": np.ascontiguousarray(h[b, r * SL:(r + 1) * SL, :]),
            "wq": wq_b, "wk": wk_b, "wv": wv_b,
            "wo": wo_b, "w1": w1_b, "w2": w2_b,
        }
        if has_qkv_bias:
            m["bq"], m["bk"], m["bv"] = bq, bk, bv
        if has_o_bias:
            m["bo"] = bo
        if has_ffn_bias:
            m["b1"], m["b2"] = b1, b2
        if has_ln1_aff:
            m["g1"], m["be1"] = ln1_g, ln1_b
        if has_ln2_aff:
            m["g2"], m["be2"] = ln2_g, ln2_b
        if has_mask:
            m["maskT"] = np.ascontiguousarray(mt[:, r * SL:(r + 1) * SL])
        in_maps.append(m)

    trace = bool(os.environ.get("KBENCH_TRACE"))
    tmpdir = os.environ.get("KBENCH_TMPDIR") or None
    res = run_bass_kernel_spmd(nc, in_maps, list(range(8)), trace=trace,
                               tmpdir=tmpdir)
    LAST_RESULTS = res
    out = np.stack(
        [np.concatenate([res.results[2 * b]["out"],
                         res.results[2 * b + 1]["out"]], axis=0)
         for b in range(4)], axis=0)
    return out.astype(np.float32)
